# revision 29
# baseline (speedup 1.0000x reference)
"""RNN-T JointNetwork kernel for 8 Trainium2 NeuronCores.

reference:
    combined = f[:, :, None, :] + p[:, None, :, :]   # (B,T,U,H)
    h = relu(combined)
    logits = einsum('btuh,vh->btuv', h, W) + b        # (B,T,U,V)

Shapes: f (8,256,640) p (8,64,640) W (1024,640) b (1024,) -> out (8,256,64,1024) f32.

Sharding: data-parallel over B — core i computes batch i. W/b replicated.

Per-core program (SPMD, bf16 matmuls, rel-err budget 2e-2; bf16 end-to-end
measures 4.2e-3).  The matmul stream runs at the PE hardware floor
(655,360 moving columns = 276.3us at 2.4 GHz); everything else hides
under it:
  - inputs host-transposed, host-swizzled to partition-major [128, k*X]
    (large contiguous DMA descriptors) and cast to bf16.
  - input DMAs + a zero-memset PE warm-up are emitted as raw bass
    instructions BEFORE the TileContext, so they execute during the
    framework preamble; in-Tile consumers are guarded by per-DMA
    semaphore wait-nops injected after Tile scheduling.  The warm-up
    opens the HAM clock gate (2.4 GHz) before the first real matmul.
    (Warm-up source must be memset, NOT uninitialized SBUF: garbage
    operands spike power and throttle every engine clock by 1.2x for
    the rest of the kernel.)
  - h_u[h,t] = relu(ft[h,t] + pt[h,u]) via ScalarE activation (bias = pt
    column), output bf16.
  - logits[t, u, :] via PE: out[tile] = h_u[kchunk, tslice].T @ wt[kchunk,
    vslice] accumulated over 5 k-chunks into PSUM; the first u runs
    k-outermost so each wt chunk's DMA hides behind matmuls on the
    previous chunk.  DVE adds bias while copying PSUM->SBUF as bf16;
    staged tiles are DMA'd out 1 MiB at a time, per-u (256 KiB) for the
    last group to shorten the drain tail.
  - host casts the bf16 output back to f32.
"""

import ml_dtypes
import numpy as np

import concourse.bass as bass
import concourse.mybir as mybir
import concourse.tile as tile
from concourse.bass_utils import run_bass_kernel_spmd
from concourse.vector_clock import ScopedClock

B, T, U, H, V = 8, 256, 64, 640, 1024
KC = H // 128          # 5 contraction chunks
TC = T // 128          # 2 t chunks
N_CORES = 8
UG = 4                 # u values staged per output DMA
N_WARMUP_MM = 19       # cold-rate matmuls bridging until real work is ready

_PATCHED = False


_MAX_WAITS = 1  # this walrus build rejects >1 sem-wait per instruction


def _spill_waits(nc, inst, add):
    """If `inst` carries more than _MAX_WAITS sem-waits, move the excess onto
    same-engine nops emitted (in program order) just before it."""
    si = inst.sync_info
    waits = list(si.on_wait) if si and si.on_wait else []
    if len(waits) <= _MAX_WAITS:
        return
    excess = waits[: len(waits) - _MAX_WAITS]
    inst.sync_info = mybir.SyncInfo(
        on_wait=waits[len(waits) - _MAX_WAITS :],
        on_update=list(si.on_update or []),
    )
    for i in range(0, len(excess), _MAX_WAITS):
        nop = mybir.InstNoOp(name=f"{inst.name}_spillw{i}", ins=[], outs=[])
        nop.engine = inst.engine
        nop.sync_info = mybir.SyncInfo(
            on_wait=excess[i : i + _MAX_WAITS], on_update=[]
        )
        nc.register_instruction(nop, overwrite=True)
        add(nop)


def _patch_tile_drain():
    """This walrus build's setupSyncWait rejects instructions carrying more
    than one sem-wait.  Tile freely emits several per instruction, so (a)
    split excess waits onto same-engine nops as instructions are committed
    into basic blocks, and (b) do the same for the end-of-kernel drain."""
    global _PATCHED
    if _PATCHED:
        return
    _PATCHED = True

    orig_add = tile.TileContext._add_instruction

    def _add_instruction(self, inst):
        _spill_waits(self.nc, inst, lambda n: orig_add(self, n))
        orig_add(self, inst)

    tile.TileContext._add_instruction = _add_instruction

    def _drain_and_barrier(self, tick_clock, wait_clock):
        nc = self.nc
        probe = nc.sync.nop(nofuse=True, hint="drain_wait_probe")
        wait_clock.add_sem_waits(
            probe.ins, ScopedClock({None: tick_clock.global_clock})
        )
        si = probe.ins.sync_info
        waits = list(si.on_wait) if si and si.on_wait else []
        if len(waits) > _MAX_WAITS:
            probe.ins.sync_info = mybir.SyncInfo(
                on_wait=waits[:_MAX_WAITS], on_update=list(si.on_update or [])
            )
            rest = waits[_MAX_WAITS:]
            for i in range(0, len(rest), _MAX_WAITS):
                extra = nc.sync.nop(nofuse=True, hint=f"drain_wait_{i}")
                extra.ins.sync_info = mybir.SyncInfo(
                    on_wait=rest[i : i + _MAX_WAITS], on_update=[]
                )
        nc.sync.drain()
        nc.all_engine_barrier()
        assert self.sems is not None
        popped = nc._tile_sem_poison_stack.pop()
        assert popped is self._sem_poison
        nc.clear_and_free_semaphores(list(self.sems.allocated().values()))
        nc.all_engine_barrier()

    tile.TileContext._drain_and_barrier = _drain_and_barrier


def build_program():
    """One SPMD NeuronCore program: (T,U,V) joint-network slice for one batch."""
    _patch_tile_drain()
    nc = bass.Bass()
    f32 = mybir.dt.float32
    bf16 = mybir.dt.bfloat16

    # Inputs arrive host-swizzled to partition-major [128, k*X] so every
    # input DMA is 128 large contiguous descriptors (the (k p) x -> p k x
    # rearrange on the DMA itself generates 128*KC small descriptors and
    # crawls at ~80 GB/s).
    ft = nc.dram_tensor("ft", [128, KC * T], bf16, kind="ExternalInput")
    pt = nc.dram_tensor("pt", [128, KC * U], bf16, kind="ExternalInput")
    wt = nc.dram_tensor("wt", [128, KC * V], bf16, kind="ExternalInput")
    bias = nc.dram_tensor("bias", [128, V], bf16, kind="ExternalInput")
    out = nc.dram_tensor("out", [T, U, V], bf16, kind="ExternalOutput")

    # ── pre-Tile prefetch + PE warm-up ──────────────────────────────────
    # Everything here lands on the engine queues right after the Bass-init
    # barrier (~5.7us), well before the Tile preamble finishes, so input
    # data is in flight (and the PE HAM clock gate open) by the time the
    # main loop starts.  Consumers inside the Tile region are guarded by
    # per-DMA semaphore waits injected after Tile scheduling (the Tile
    # block simulator cannot see these out-of-block increments).
    ft_sb = nc.alloc_sbuf_tensor("ft_sb", [128, KC, T], bf16).ap()
    pt_sb = nc.alloc_sbuf_tensor("pt_sb", [128, KC, U], bf16).ap()
    wt_sb = nc.alloc_sbuf_tensor("wt_sb", [128, KC, V], bf16).ap()
    bias_sb = nc.alloc_sbuf_tensor("bias_sb", [128, V], bf16).ap()
    warm_sb = nc.alloc_sbuf_tensor("warm_sb", [128, 320], bf16).ap()
    s_pt = nc.alloc_semaphore("s_pt")
    s_ft = nc.alloc_semaphore("s_ft")
    s_wt = [nc.alloc_semaphore(f"s_wt{k}") for k in range(KC)]
    s_bias = nc.alloc_semaphore("s_bias")

    # pt/ft first on the sync ring (it ramps up fastest and gates the first
    # activation); the first wt chunks go out in parallel on the scalar ring
    # so each chunk's completion beats the matmul pipeline's arrival.
    nc.sync.dma_start(pt_sb[:], pt[:]).then_inc(s_pt, 16)
    nc.sync.dma_start(ft_sb[:], ft[:]).then_inc(s_ft, 16)
    for k, eng in [(0, nc.scalar), (1, nc.scalar), (2, nc.sync),
                   (3, nc.sync), (4, nc.sync)]:
        eng.dma_start(
            wt_sb[:, k, :], wt[:, k * V : (k + 1) * V]
        ).then_inc(s_wt[k], 16)
    nc.scalar.dma_start(bias_sb[:], bias[:]).then_inc(s_bias, 16)

    # Dummy activation: forces walrus to place the ~1.3us ACT_TABLE_LOAD
    # here, concurrent with the input DMAs, instead of in front of the
    # first real activation.
    nc.scalar.activation(
        warm_sb[:, :1], warm_sb[:, :1], mybir.ActivationFunctionType.Relu
    )

    # PE warm-up (result unread, bank reused by Tile afterwards — safe: PE
    # executes in program order).  The tile MUST be memset first: matmuls on
    # uninitialized SBUF (random bit patterns) spike power draw enough to
    # kick the chip into its throttled power state for the whole kernel
    # (measured: every engine clock drops by 1.2x).
    ws_sem = nc.alloc_semaphore("ws_sem")
    nc.gpsimd.memset(warm_sb[:], 0.0).then_inc(ws_sem, 1)
    psum_base_save = nc.psum_base
    warm_ps = nc.alloc_psum_tensor("warm_ps", [64, 320], f32).ap()
    for w in range(N_WARMUP_MM):
        mm_w = nc.tensor.matmul(
            warm_ps[:], warm_sb[:, :64], warm_sb[:], start=True, stop=True
        )
        if w == 0:
            mm_w.wait_op(ws_sem, 1, "sem-ge")
    nc.psum_base = psum_base_save

    # (target mybir instruction, semaphore, threshold) — resolved into
    # wait-nops inserted just before each target after Tile scheduling.
    guards = []

    with tile.TileContext(nc) as tc:
        with (
            tc.tile_pool(name="h", bufs=4) as hpool,
            tc.tile_pool(name="stage", bufs=3) as spool,
            tc.tile_pool(name="psum", bufs=8, space="PSUM") as ppool,
        ):
            for u0 in range(0, U, UG):
                last_group = u0 + UG >= U
                stages = [spool.tile([128, UG, V], bf16, tag=f"st{t_}",
                                     name=f"stage{t_}_{u0}")
                          for t_ in range(TC)]
                for j in range(UG):
                    u = u0 + j
                    h_u = hpool.tile([128, KC, T], bf16, tag="h")
                    for k in range(KC):
                        act = nc.scalar.activation(
                            h_u[:, k, :],
                            ft_sb[:, k, :],
                            mybir.ActivationFunctionType.Relu,
                            bias=pt_sb[:, k, u : u + 1],
                        )
                        if u == 0 and k == 0:
                            guards.append((act.ins, s_pt, 16))
                            guards.append((act.ins, s_ft, 16))
                    if u == 0:
                        # k-outermost for the very first u: each wt chunk's
                        # DMA completion hides behind ~0.9us of matmuls on
                        # the previous chunk, instead of stalling the PE.
                        psums = {
                            (t_, h_): ppool.tile([128, 512], f32, tag="ps",
                                                 name=f"ps0_{t_}_{h_}")
                            for t_ in range(TC) for h_ in range(2)
                        }
                        for k in range(KC):
                            first = True
                            for t_ in range(TC):
                                lhsT = h_u[:, k, t_ * 128 : (t_ + 1) * 128]
                                for h_ in range(2):
                                    mm = nc.tensor.matmul(
                                        psums[t_, h_][:],
                                        lhsT,
                                        wt_sb[:, k,
                                              h_ * 512 : (h_ + 1) * 512],
                                        start=(k == 0),
                                        stop=(k == KC - 1),
                                    )
                                    if first:
                                        guards.append((mm.ins, s_wt[k], 16))
                                        first = False
                        for t_ in range(TC):
                            for h_ in range(2):
                                sl = slice(h_ * 512, (h_ + 1) * 512)
                                add = nc.vector.tensor_add(
                                    stages[t_][:, j, sl],
                                    psums[t_, h_][:],
                                    bias_sb[:, sl],
                                )
                                if t_ == 0 and h_ == 0:
                                    guards.append((add.ins, s_bias, 16))
                        continue
                    for t_ in range(TC):
                        psums = [ppool.tile([128, 512], f32, tag="ps",
                                            name=f"ps{u}_{t_}_{h_}")
                                 for h_ in range(2)]
                        for k in range(KC):
                            lhsT = h_u[:, k, t_ * 128 : (t_ + 1) * 128]
                            for h_ in range(2):
                                nc.tensor.matmul(
                                    psums[h_][:],
                                    lhsT,
                                    wt_sb[:, k, h_ * 512 : (h_ + 1) * 512],
                                    start=(k == 0),
                                    stop=(k == KC - 1),
                                )
                        for h_ in range(2):
                            sl = slice(h_ * 512, (h_ + 1) * 512)
                            nc.vector.tensor_add(
                                stages[t_][:, j, sl],
                                psums[h_][:],
                                bias_sb[:, sl],
                            )
                    if last_group:
                        # per-u output DMA at the end: the tail after the
                        # final matmul only has to drain 256 KiB, not 1 MiB
                        for t_ in range(TC):
                            nc.sync.dma_start(
                                out[t_ * 128 : (t_ + 1) * 128,
                                    u : u + 1, :],
                                stages[t_][:, j : j + 1, :],
                            )
                if not last_group:
                    for t_ in range(TC):
                        nc.sync.dma_start(
                            out[t_ * 128 : (t_ + 1) * 128, u0 : u0 + UG, :],
                            stages[t_][:],
                        )

    # Inject the prefetch guards now that Tile scheduling is done: a wait-nop
    # on the consumer's engine immediately before the first consumer of each
    # prefetched tensor (Tile's block simulator would deadlock on waits whose
    # increments happen outside the block, so they cannot be emitted inline).
    eng_ns = {
        mybir.EngineType.PE: nc.tensor,
        mybir.EngineType.Activation: nc.scalar,
        mybir.EngineType.DVE: nc.vector,
    }
    fn = nc.m.functions[0]

    def _find(inst):
        for b in fn.blocks:
            for idx, x in enumerate(b.instructions):
                if x is inst:
                    return b, idx
        raise KeyError(inst.name)

    for target, sem, val in guards:
        nopi = eng_ns[target.engine].nop(nofuse=True, hint="prefetch_guard")
        nopi.wait_op(sem, val, "sem-ge")
        src_blk, src_idx = _find(nopi.ins)
        del src_blk.instructions[src_idx]
        dst_blk, dst_idx = _find(target)
        dst_blk.instructions.insert(dst_idx, nopi.ins)
    return nc


def _swz(xT, last):
    """(H, last) -> partition-major (128, KC*last): row p holds chunks
    k=0..KC-1 of H-rows {k*128+p}, each contiguous."""
    bf = ml_dtypes.bfloat16
    return np.ascontiguousarray(
        xT.reshape(KC, 128, last).transpose(1, 0, 2).reshape(128, KC * last)
    ).astype(bf)


def prepare_inputs(f, p, W, b):
    """Host-side shard + layout prep: per-core bf16 in_maps."""
    f = np.asarray(f, np.float32)
    p = np.asarray(p, np.float32)
    W = np.asarray(W, np.float32)
    b = np.asarray(b, np.float32)
    bf = ml_dtypes.bfloat16
    wt = _swz(np.ascontiguousarray(W.T), V)                     # (128, KC*V)
    bias = np.ascontiguousarray(np.broadcast_to(b, (128, V))).astype(bf)
    return [
        {
            "ft": _swz(np.ascontiguousarray(f[i].T), T),        # (128, KC*T)
            "pt": _swz(np.ascontiguousarray(p[i].T), U),        # (128, KC*U)
            "wt": wt,
            "bias": bias,
        }
        for i in range(N_CORES)
    ]


def kernel(f, p, W, b):
    nc = build_program()
    in_maps = prepare_inputs(f, p, W, b)
    res = run_bass_kernel_spmd(nc, in_maps, list(range(N_CORES)))
    out = np.stack([res.results[i]["out"] for i in range(N_CORES)], axis=0)
    return out.astype(np.float32)


# revision 30
# speedup vs baseline: 1.0020x; 1.0020x over previous
"""RNN-T JointNetwork kernel for 8 Trainium2 NeuronCores.

reference:
    combined = f[:, :, None, :] + p[:, None, :, :]   # (B,T,U,H)
    h = relu(combined)
    logits = einsum('btuh,vh->btuv', h, W) + b        # (B,T,U,V)

Shapes: f (8,256,640) p (8,64,640) W (1024,640) b (1024,) -> out (8,256,64,1024) f32.

Sharding: data-parallel over B — core i computes batch i. W/b replicated.

Per-core program (SPMD, bf16 matmuls, rel-err budget 2e-2; bf16 end-to-end
measures 4.2e-3).  The matmul stream runs at the PE hardware floor
(655,360 moving columns = 276.3us at 2.4 GHz); everything else hides
under it:
  - inputs host-transposed, host-swizzled to partition-major [128, k*X]
    (large contiguous DMA descriptors) and cast to bf16.
  - input DMAs + a zero-memset PE warm-up are emitted as raw bass
    instructions BEFORE the TileContext, so they execute during the
    framework preamble; in-Tile consumers are guarded by per-DMA
    semaphore wait-nops injected after Tile scheduling.  The warm-up
    opens the HAM clock gate (2.4 GHz) before the first real matmul.
    (Warm-up source must be memset, NOT uninitialized SBUF: garbage
    operands spike power and throttle every engine clock by 1.2x for
    the rest of the kernel.)
  - h_u[h,t] = relu(ft[h,t] + pt[h,u]) via ScalarE activation (bias = pt
    column), output bf16.
  - logits[t, u, :] via PE: out[tile] = h_u[kchunk, tslice].T @ wt[kchunk,
    vslice] accumulated over 5 k-chunks into PSUM; the first u runs
    k-outermost so each wt chunk's DMA hides behind matmuls on the
    previous chunk.  DVE adds bias while copying PSUM->SBUF as bf16;
    staged tiles are DMA'd out 1 MiB at a time, per-u (256 KiB) for the
    last group to shorten the drain tail.
  - host casts the bf16 output back to f32.
"""

import ml_dtypes
import numpy as np

import concourse.bass as bass
import concourse.mybir as mybir
import concourse.tile as tile
from concourse.bass_utils import run_bass_kernel_spmd
from concourse.vector_clock import ScopedClock

B, T, U, H, V = 8, 256, 64, 640, 1024
KC = H // 128          # 5 contraction chunks
TC = T // 128          # 2 t chunks
N_CORES = 8
UG = 4                 # u values staged per output DMA
N_WARMUP_MM = 19       # cold-rate matmuls bridging until real work is ready

_PATCHED = False


_MAX_WAITS = 1  # this walrus build rejects >1 sem-wait per instruction


def _spill_waits(nc, inst, add):
    """If `inst` carries more than _MAX_WAITS sem-waits, move the excess onto
    same-engine nops emitted (in program order) just before it."""
    si = inst.sync_info
    waits = list(si.on_wait) if si and si.on_wait else []
    if len(waits) <= _MAX_WAITS:
        return
    excess = waits[: len(waits) - _MAX_WAITS]
    inst.sync_info = mybir.SyncInfo(
        on_wait=waits[len(waits) - _MAX_WAITS :],
        on_update=list(si.on_update or []),
    )
    for i in range(0, len(excess), _MAX_WAITS):
        nop = mybir.InstNoOp(name=f"{inst.name}_spillw{i}", ins=[], outs=[])
        nop.engine = inst.engine
        nop.sync_info = mybir.SyncInfo(
            on_wait=excess[i : i + _MAX_WAITS], on_update=[]
        )
        nc.register_instruction(nop, overwrite=True)
        add(nop)


def _patch_tile_drain():
    """This walrus build's setupSyncWait rejects instructions carrying more
    than one sem-wait.  Tile freely emits several per instruction, so (a)
    split excess waits onto same-engine nops as instructions are committed
    into basic blocks, and (b) do the same for the end-of-kernel drain."""
    global _PATCHED
    if _PATCHED:
        return
    _PATCHED = True

    orig_add = tile.TileContext._add_instruction

    def _add_instruction(self, inst):
        _spill_waits(self.nc, inst, lambda n: orig_add(self, n))
        orig_add(self, inst)

    tile.TileContext._add_instruction = _add_instruction

    def _drain_and_barrier(self, tick_clock, wait_clock):
        nc = self.nc
        probe = nc.sync.nop(nofuse=True, hint="drain_wait_probe")
        wait_clock.add_sem_waits(
            probe.ins, ScopedClock({None: tick_clock.global_clock})
        )
        si = probe.ins.sync_info
        waits = list(si.on_wait) if si and si.on_wait else []
        if len(waits) > _MAX_WAITS:
            probe.ins.sync_info = mybir.SyncInfo(
                on_wait=waits[:_MAX_WAITS], on_update=list(si.on_update or [])
            )
            rest = waits[_MAX_WAITS:]
            for i in range(0, len(rest), _MAX_WAITS):
                extra = nc.sync.nop(nofuse=True, hint=f"drain_wait_{i}")
                extra.ins.sync_info = mybir.SyncInfo(
                    on_wait=rest[i : i + _MAX_WAITS], on_update=[]
                )
        nc.sync.drain()
        nc.all_engine_barrier()
        assert self.sems is not None
        popped = nc._tile_sem_poison_stack.pop()
        assert popped is self._sem_poison
        nc.clear_and_free_semaphores(list(self.sems.allocated().values()))
        nc.all_engine_barrier()

    tile.TileContext._drain_and_barrier = _drain_and_barrier


def build_program():
    """One SPMD NeuronCore program: (T,U,V) joint-network slice for one batch."""
    _patch_tile_drain()
    nc = bass.Bass()
    f32 = mybir.dt.float32
    bf16 = mybir.dt.bfloat16

    # Inputs arrive host-swizzled to partition-major [128, k*X] so every
    # input DMA is 128 large contiguous descriptors (the (k p) x -> p k x
    # rearrange on the DMA itself generates 128*KC small descriptors and
    # crawls at ~80 GB/s).
    ft = nc.dram_tensor("ft", [128, KC * T], bf16, kind="ExternalInput")
    pt = nc.dram_tensor("pt", [128, KC * U], bf16, kind="ExternalInput")
    wt = nc.dram_tensor("wt", [128, KC * V], bf16, kind="ExternalInput")
    bias = nc.dram_tensor("bias", [128, V], bf16, kind="ExternalInput")
    out = nc.dram_tensor("out", [T, U, V], bf16, kind="ExternalOutput")

    # ── pre-Tile prefetch + PE warm-up ──────────────────────────────────
    # Everything here lands on the engine queues right after the Bass-init
    # barrier (~5.7us), well before the Tile preamble finishes, so input
    # data is in flight (and the PE HAM clock gate open) by the time the
    # main loop starts.  Consumers inside the Tile region are guarded by
    # per-DMA semaphore waits injected after Tile scheduling (the Tile
    # block simulator cannot see these out-of-block increments).
    ft_sb = nc.alloc_sbuf_tensor("ft_sb", [128, KC, T], bf16).ap()
    pt_sb = nc.alloc_sbuf_tensor("pt_sb", [128, KC, U], bf16).ap()
    wt_sb = nc.alloc_sbuf_tensor("wt_sb", [128, KC, V], bf16).ap()
    bias_sb = nc.alloc_sbuf_tensor("bias_sb", [128, V], bf16).ap()
    warm_sb = nc.alloc_sbuf_tensor("warm_sb", [128, 320], bf16).ap()
    s_pt = nc.alloc_semaphore("s_pt")
    s_ft = nc.alloc_semaphore("s_ft")
    s_wt = [nc.alloc_semaphore(f"s_wt{k}") for k in range(KC)]
    s_bias = nc.alloc_semaphore("s_bias")

    # pt/ft first on the sync ring (it ramps up fastest and gates the first
    # activation); the first wt chunks go out in parallel on the scalar ring
    # so each chunk's completion beats the matmul pipeline's arrival.
    nc.sync.dma_start(pt_sb[:], pt[:]).then_inc(s_pt, 16)
    nc.sync.dma_start(ft_sb[:], ft[:]).then_inc(s_ft, 16)
    for k, eng in [(0, nc.scalar), (1, nc.scalar), (2, nc.sync),
                   (3, nc.sync), (4, nc.sync)]:
        eng.dma_start(
            wt_sb[:, k, :], wt[:, k * V : (k + 1) * V]
        ).then_inc(s_wt[k], 16)
    nc.scalar.dma_start(bias_sb[:], bias[:]).then_inc(s_bias, 16)

    # Dummy activation: forces walrus to place the ~1.3us ACT_TABLE_LOAD
    # here, concurrent with the input DMAs, instead of in front of the
    # first real activation.
    nc.scalar.activation(
        warm_sb[:, :1], warm_sb[:, :1], mybir.ActivationFunctionType.Relu
    )

    # PE warm-up (result unread, bank reused by Tile afterwards — safe: PE
    # executes in program order).  The tile MUST be memset first: matmuls on
    # uninitialized SBUF (random bit patterns) spike power draw enough to
    # kick the chip into its throttled power state for the whole kernel
    # (measured: every engine clock drops by 1.2x).
    ws_sem = nc.alloc_semaphore("ws_sem")
    nc.gpsimd.memset(warm_sb[:], 0.0).then_inc(ws_sem, 1)
    psum_base_save = nc.psum_base
    warm_ps = nc.alloc_psum_tensor("warm_ps", [64, 320], f32).ap()
    for w in range(N_WARMUP_MM):
        mm_w = nc.tensor.matmul(
            warm_ps[:], warm_sb[:, :64], warm_sb[:], start=True, stop=True
        )
        if w == 0:
            mm_w.wait_op(ws_sem, 1, "sem-ge")
    nc.psum_base = psum_base_save

    # (target mybir instruction, semaphore, threshold) — resolved into
    # wait-nops inserted just before each target after Tile scheduling.
    guards = []

    with tile.TileContext(nc) as tc:
        with (
            tc.tile_pool(name="h", bufs=4) as hpool,
            tc.tile_pool(name="stage", bufs=3) as spool,
            tc.tile_pool(name="psum", bufs=8, space="PSUM") as ppool,
        ):
            for u0 in range(0, U, UG):
                last_group = u0 + UG >= U
                stages = [spool.tile([128, UG, V], bf16, tag=f"st{t_}",
                                     name=f"stage{t_}_{u0}")
                          for t_ in range(TC)]
                for j in range(UG):
                    u = u0 + j
                    h_u = hpool.tile([128, KC, T], bf16, tag="h")
                    for k in range(KC):
                        act = nc.scalar.activation(
                            h_u[:, k, :],
                            ft_sb[:, k, :],
                            mybir.ActivationFunctionType.Relu,
                            bias=pt_sb[:, k, u : u + 1],
                        )
                        if u == 0 and k == 0:
                            guards.append((act.ins, s_pt, 16))
                            guards.append((act.ins, s_ft, 16))
                    if u == 0:
                        # k-outermost for the very first u: each wt chunk's
                        # DMA completion hides behind ~0.9us of matmuls on
                        # the previous chunk, instead of stalling the PE.
                        psums = {
                            (t_, h_): ppool.tile([128, 512], f32, tag="ps",
                                                 name=f"ps0_{t_}_{h_}")
                            for t_ in range(TC) for h_ in range(2)
                        }
                        for k in range(KC):
                            first = True
                            for t_ in range(TC):
                                lhsT = h_u[:, k, t_ * 128 : (t_ + 1) * 128]
                                for h_ in range(2):
                                    mm = nc.tensor.matmul(
                                        psums[t_, h_][:],
                                        lhsT,
                                        wt_sb[:, k,
                                              h_ * 512 : (h_ + 1) * 512],
                                        start=(k == 0),
                                        stop=(k == KC - 1),
                                    )
                                    if first:
                                        guards.append((mm.ins, s_wt[k], 16))
                                        first = False
                        for t_ in range(TC):
                            for h_ in range(2):
                                sl = slice(h_ * 512, (h_ + 1) * 512)
                                add = nc.vector.tensor_add(
                                    stages[t_][:, j, sl],
                                    psums[t_, h_][:],
                                    bias_sb[:, sl],
                                )
                                if t_ == 0 and h_ == 0:
                                    guards.append((add.ins, s_bias, 16))
                        continue
                    for t_ in range(TC):
                        psums = [ppool.tile([128, 512], f32, tag="ps",
                                            name=f"ps{u}_{t_}_{h_}")
                                 for h_ in range(2)]
                        for k in range(KC):
                            lhsT = h_u[:, k, t_ * 128 : (t_ + 1) * 128]
                            for h_ in range(2):
                                nc.tensor.matmul(
                                    psums[h_][:],
                                    lhsT,
                                    wt_sb[:, k, h_ * 512 : (h_ + 1) * 512],
                                    start=(k == 0),
                                    stop=(k == KC - 1),
                                )
                        for h_ in range(2):
                            sl = slice(h_ * 512, (h_ + 1) * 512)
                            nc.vector.tensor_add(
                                stages[t_][:, j, sl],
                                psums[h_][:],
                                bias_sb[:, sl],
                            )
                            if u == U - 1:
                                # final u: DMA each half as soon as its add
                                # lands, so the last DMA's receipt (which
                                # gates the drain) starts ~0.7us earlier
                                nc.sync.dma_start(
                                    out[t_ * 128 : (t_ + 1) * 128,
                                        u : u + 1, sl],
                                    stages[t_][:, j : j + 1, sl],
                                )
                    if last_group and u != U - 1:
                        # per-u output DMA at the end: the tail after the
                        # final matmul only has to drain 256 KiB, not 1 MiB
                        for t_ in range(TC):
                            nc.sync.dma_start(
                                out[t_ * 128 : (t_ + 1) * 128,
                                    u : u + 1, :],
                                stages[t_][:, j : j + 1, :],
                            )
                if not last_group:
                    for t_ in range(TC):
                        nc.sync.dma_start(
                            out[t_ * 128 : (t_ + 1) * 128, u0 : u0 + UG, :],
                            stages[t_][:],
                        )

    # Inject the prefetch guards now that Tile scheduling is done: a wait-nop
    # on the consumer's engine immediately before the first consumer of each
    # prefetched tensor (Tile's block simulator would deadlock on waits whose
    # increments happen outside the block, so they cannot be emitted inline).
    eng_ns = {
        mybir.EngineType.PE: nc.tensor,
        mybir.EngineType.Activation: nc.scalar,
        mybir.EngineType.DVE: nc.vector,
    }
    fn = nc.m.functions[0]

    def _find(inst):
        for b in fn.blocks:
            for idx, x in enumerate(b.instructions):
                if x is inst:
                    return b, idx
        raise KeyError(inst.name)

    for target, sem, val in guards:
        nopi = eng_ns[target.engine].nop(nofuse=True, hint="prefetch_guard")
        nopi.wait_op(sem, val, "sem-ge")
        src_blk, src_idx = _find(nopi.ins)
        del src_blk.instructions[src_idx]
        dst_blk, dst_idx = _find(target)
        dst_blk.instructions.insert(dst_idx, nopi.ins)
    return nc


def _swz(xT, last):
    """(H, last) -> partition-major (128, KC*last): row p holds chunks
    k=0..KC-1 of H-rows {k*128+p}, each contiguous."""
    bf = ml_dtypes.bfloat16
    return np.ascontiguousarray(
        xT.reshape(KC, 128, last).transpose(1, 0, 2).reshape(128, KC * last)
    ).astype(bf)


def prepare_inputs(f, p, W, b):
    """Host-side shard + layout prep: per-core bf16 in_maps."""
    f = np.asarray(f, np.float32)
    p = np.asarray(p, np.float32)
    W = np.asarray(W, np.float32)
    b = np.asarray(b, np.float32)
    bf = ml_dtypes.bfloat16
    wt = _swz(np.ascontiguousarray(W.T), V)                     # (128, KC*V)
    bias = np.ascontiguousarray(np.broadcast_to(b, (128, V))).astype(bf)
    return [
        {
            "ft": _swz(np.ascontiguousarray(f[i].T), T),        # (128, KC*T)
            "pt": _swz(np.ascontiguousarray(p[i].T), U),        # (128, KC*U)
            "wt": wt,
            "bias": bias,
        }
        for i in range(N_CORES)
    ]


def kernel(f, p, W, b):
    nc = build_program()
    in_maps = prepare_inputs(f, p, W, b)
    res = run_bass_kernel_spmd(nc, in_maps, list(range(N_CORES)))
    out = np.stack([res.results[i]["out"] for i in range(N_CORES)], axis=0)
    return out.astype(np.float32)
